# revision 9
# baseline (speedup 1.0000x reference)
"""TRN2 Bass kernel for nn_Attention_m_17815524344494.

Multi-head attention over [B=8, M=4, P=512, H=768], nh=12, hs=64.
Sharding: data-parallel over batch B -> one batch element per NeuronCore (8 cores).

Per-core dataflow (T = M*P = 2048 tokens, all matmuls in float32r ~ TF32):
  1. x [2048,768] -> PE-transpose -> xT (feature-major) per modality
  2. qT = Wq^T xT, kT = Wk^T xT (feature-major), v = x Wv (token-major,
     augmented with a ones column per head for free softmax sums)
  3. per (modality, head): scoresT = kT^T q (keys on partitions),
     eT = exp(scoresT/8) via ScalarE, ctxT_unnorm/sums = v_aug^T eT,
     sums broadcast via K=1 outer-product matmul, normalize on VectorE
  4. out = ctxT^T Wo (token-major), DMA to DRAM

Biases are zeros per the problem spec; a numpy fallback handles the
(never exercised) nonzero-bias case.
"""

from contextlib import ExitStack

import numpy as np

import concourse.bass as bass
import concourse.mybir as mybir
from concourse import bacc, bass_utils
from concourse.tile import TileContext
from concourse.masks import make_identity

F32 = mybir.dt.float32
F32R = mybir.dt.float32r
AF = mybir.ActivationFunctionType
ALU = mybir.AluOpType

B, M, PM, H = 8, 4, 512, 768
NH, HS = 12, 64
T = M * PM          # 2048 tokens per core
HC = H // 128       # 6 hidden chunks
TCM = PM // 128     # 4 token chunks per modality


def _emit(tc, ctx):
    nc = tc.nc

    x_ap = nc.dram_tensor("x", [T, H], F32, kind="ExternalInput").ap()
    wq_ap = nc.dram_tensor("wq", [H, H], F32, kind="ExternalInput").ap()
    wk_ap = nc.dram_tensor("wk", [H, H], F32, kind="ExternalInput").ap()
    wv_ap = nc.dram_tensor("wv", [H, H], F32, kind="ExternalInput").ap()
    wo_ap = nc.dram_tensor("wo", [H, H], F32, kind="ExternalInput").ap()
    out_ap = nc.dram_tensor("out", [T, H], F32, kind="ExternalOutput").ap()

    const = ctx.enter_context(tc.tile_pool(name="const", bufs=1))
    wpool = ctx.enter_context(tc.tile_pool(name="w", bufs=1))
    xin = ctx.enter_context(tc.tile_pool(name="xin", bufs=2))
    xtp = ctx.enter_context(tc.tile_pool(name="xt", bufs=1))
    qpool = ctx.enter_context(tc.tile_pool(name="q", bufs=2))
    kpool = ctx.enter_context(tc.tile_pool(name="k", bufs=2))
    vpool = ctx.enter_context(tc.tile_pool(name="v", bufs=2))
    epool = ctx.enter_context(tc.tile_pool(name="e", bufs=5))
    spool = ctx.enter_context(tc.tile_pool(name="s", bufs=2))
    rpool = ctx.enter_context(tc.tile_pool(name="r", bufs=2))
    cpool = ctx.enter_context(tc.tile_pool(name="ctx", bufs=1))
    opool = ctx.enter_context(tc.tile_pool(name="o", bufs=2))
    ps_big = ctx.enter_context(tc.tile_pool(name="ps_big", bufs=5, space="PSUM"))
    ps_c = ctx.enter_context(tc.tile_pool(name="ps_c", bufs=2, space="PSUM"))
    ps_b = ctx.enter_context(tc.tile_pool(name="ps_b", bufs=1, space="PSUM"))

    # f32r tiles can't be written by memset/affine_select directly (no
    # f32r rounding on those ISA paths); stage in f32 and copy via DVE.
    id_stage = const.tile([128, 128], F32)
    make_identity(nc, id_stage[:])
    identity = const.tile([128, 128], F32R)
    nc.vector.tensor_copy(identity[:], id_stage[:])
    ones_stage = const.tile([128, 64], F32)
    nc.gpsimd.memset(ones_stage[:], 1.0)
    ones64 = const.tile([1, 64], F32R)
    nc.vector.tensor_copy(ones64[:], ones_stage[:1, :])
    onescol = const.tile([128, NH * TCM], F32R)
    nc.vector.tensor_copy(onescol[:], ones_stage[:, :NH * TCM])

    w_tiles = {}
    for name, ap in (("wq", wq_ap), ("wk", wk_ap), ("wv", wv_ap), ("wo", wo_ap)):
        t = wpool.tile([128, HC, H], F32R, tag=name)
        nc.gpsimd.dma_start(t[:], ap.rearrange("(kc p) j -> p kc j", p=128))
        w_tiles[name] = t

    for m in range(M):
        # ---- Phase A: load x tiles for this modality, transpose to xT ----
        xt = xtp.tile([128, HC, PM], F32R, tag="xt")
        for ti in range(TCM):
            xtile = xin.tile([128, H], F32R, tag="xin")
            row0 = (m * TCM + ti) * 128
            nc.gpsimd.dma_start(xtile[:], x_ap[row0:row0 + 128, :])
            for hc in range(HC):
                pst = ps_big.tile([128, 512], F32R, tag="ps_big")
                nc.tensor.transpose(
                    pst[:, :128], xtile[:, hc * 128:(hc + 1) * 128], identity[:]
                )
                nc.vector.tensor_copy(
                    xt[:, hc, ti * 128:(ti + 1) * 128], pst[:, :128]
                )

        # ---- Phase B: projections ----
        qt = qpool.tile([128, HC, PM], F32R, tag="q")
        kt = kpool.tile([128, HC, PM], F32R, tag="k")
        for dst, w in ((qt, w_tiles["wq"]), (kt, w_tiles["wk"])):
            for jc in range(HC):
                ps = ps_big.tile([128, 512], F32, tag="ps_big")
                for kc in range(HC):
                    nc.tensor.matmul(
                        ps[:],
                        w[:, kc, jc * 128:(jc + 1) * 128],
                        xt[:, kc, :],
                        start=(kc == 0),
                        stop=(kc == HC - 1),
                    )
                nc.vector.tensor_copy(dst[:, jc, :], ps[:])

        vt = vpool.tile([128, TCM, NH, HS + 1], F32R, tag="v")
        nc.vector.tensor_copy(
            vt[:, :, :, HS],
            onescol[:].rearrange("p (t h) -> p t h", t=TCM),
        )
        for ti in range(TCM):
            for nn in range(2):
                ps = ps_big.tile([128, 512], F32, tag="ps_big")
                for kc in range(HC):
                    nc.tensor.matmul(
                        ps[:, :384],
                        xt[:, kc, ti * 128:(ti + 1) * 128],
                        w_tiles["wv"][:, kc, nn * 384:(nn + 1) * 384],
                        start=(kc == 0),
                        stop=(kc == HC - 1),
                    )
                nc.vector.tensor_copy(
                    vt[:, ti, nn * 6:(nn + 1) * 6, :HS],
                    ps[:, :384].rearrange("p (h c) -> p h c", c=HS),
                )

        # ---- Phase C: attention per head ----
        ctxt = cpool.tile([128, HC, PM], F32R, tag="ctx")
        for h in range(NH):
            hc, hr = h // 2, (h % 2) * 64
            qh = qt[hr:hr + 64, hc, :]
            ets = []
            for jc in range(TCM):
                ps_sc = ps_big.tile([128, 512], F32, tag="ps_big")
                nc.tensor.matmul(
                    ps_sc[:],
                    kt[hr:hr + 64, hc, jc * 128:(jc + 1) * 128],
                    qh,
                    start=True,
                    stop=True,
                )
                et = epool.tile([128, 512], F32R, tag="e")
                nc.scalar.activation(et[:], ps_sc[:], AF.Exp, scale=0.125)
                ets.append(et)
            psc = ps_c.tile([HS + 1, 512], F32, tag="ps_c")
            for jc in range(TCM):
                nc.tensor.matmul(
                    psc[:],
                    vt[:, jc, h, :],
                    ets[jc][:],
                    start=(jc == 0),
                    stop=(jc == TCM - 1),
                )
            ssb = spool.tile([1, 512], F32R, tag="s")
            nc.vector.tensor_copy(ssb[:], psc[HS:HS + 1, :])
            psb = ps_b.tile([64, 512], F32, tag="ps_b")
            nc.tensor.matmul(psb[:], ones64[:], ssb[:], start=True, stop=True)
            rsb = rpool.tile([64, 512], F32, tag="r")
            nc.vector.reciprocal(rsb[:], psb[:])
            nc.vector.tensor_tensor(
                ctxt[hr:hr + 64, hc, :], psc[:HS, :], rsb[:], ALU.mult
            )

        # ---- Phase D: output projection ----
        for ti in range(TCM):
            osb = opool.tile([128, H], F32, tag="o")
            for nn in range(2):
                ps = ps_big.tile([128, 512], F32, tag="ps_big")
                for cc in range(HC):
                    nc.tensor.matmul(
                        ps[:, :384],
                        ctxt[:, cc, ti * 128:(ti + 1) * 128],
                        w_tiles["wo"][:, cc, nn * 384:(nn + 1) * 384],
                        start=(cc == 0),
                        stop=(cc == HC - 1),
                    )
                nc.vector.tensor_copy(osb[:, nn * 384:(nn + 1) * 384], ps[:, :384])
            row0 = (m * TCM + ti) * 128
            nc.sync.dma_start(out_ap[row0:row0 + 128, :], osb[:])


_NC_CACHE = {}


def build_nc():
    if "nc" not in _NC_CACHE:
        nc = bacc.Bacc("TRN2", target_bir_lowering=False, debug=False, num_devices=B)
        with TileContext(nc) as tc:
            with ExitStack() as stack:
                _emit(tc, stack)
        nc.compile()
        _NC_CACHE["nc"] = nc
    return _NC_CACHE["nc"]


def _numpy_fallback(x, Wq, bq, Wk, bk, Wv, bv, Wo, bo):
    Bb, Mm, Pp, Hh = x.shape
    xx = x.reshape(-1, Hh)
    q = (xx @ Wq + bq).reshape(Bb, Mm, Pp, NH, HS).transpose(0, 1, 3, 2, 4)
    k = (xx @ Wk + bk).reshape(Bb, Mm, Pp, NH, HS).transpose(0, 1, 3, 2, 4)
    v = (xx @ Wv + bv).reshape(Bb, Mm, Pp, NH, HS).transpose(0, 1, 3, 2, 4)
    s = np.einsum("bmnqh,bmnkh->bmnqk", q, k) / np.sqrt(HS)
    s = s - s.max(axis=-1, keepdims=True)
    e = np.exp(s)
    p = e / e.sum(axis=-1, keepdims=True)
    ctx = np.einsum("bmnqk,bmnkh->bmnqh", p, v)
    ctx = ctx.transpose(0, 1, 3, 2, 4).reshape(Bb, Mm, Pp, Hh)
    return (ctx @ Wo + bo).astype(np.float32)


def kernel(hidden_states, Wq, bq, Wk, bk, Wv, bv, Wo, bo):
    hs = np.ascontiguousarray(np.asarray(hidden_states, dtype=np.float32))
    ws = {n: np.ascontiguousarray(np.asarray(w, dtype=np.float32))
          for n, w in (("wq", Wq), ("wk", Wk), ("wv", Wv), ("wo", Wo))}
    biases = [np.asarray(b, dtype=np.float32) for b in (bq, bk, bv, bo)]
    if any(np.any(b) for b in biases):
        return _numpy_fallback(hs, ws["wq"], biases[0], ws["wk"], biases[1],
                               ws["wv"], biases[2], ws["wo"], biases[3])

    nc = build_nc()
    in_maps = [
        {"x": hs[b].reshape(T, H), **ws}
        for b in range(B)
    ]
    res = bass_utils.run_bass_kernel_spmd(nc, in_maps, core_ids=list(range(B)))
    out = np.stack([res.results[b]["out"].reshape(M, PM, H) for b in range(B)])
    return out.astype(np.float32)
